# revision 5
# baseline (speedup 1.0000x reference)
"""Trainium2 Bass kernel for nn_Attention_23424751632639.

Computation (per (b,h)):  out = tril_strict(rope(Q) @ rope(Q).T / sqrt(N)) @ V
Reformulated as chunked linear attention (exact, just reordered sums):
  out_c = QRP_c @ M_c  +  strict_mask(QRP_c @ QRP_c^T) @ V_c
  M_{c+1} = M_c + QRP_c^T @ V_c            (M is the [64,64] running state)
with QRP = rope(Q) * N**-0.25 (scale folded into the cos/sin tables, so the
score scale N**-0.5 appears automatically in both the intra and inter terms).

Feature order inside the kernel is permuted to (all even feats, all odd feats)
so RoPE needs no cross-partition ops; the permutation is contraction-invariant
and never visible in the output.

Sharding: B*H = 32 (b,h) pairs -> 4 per core across 8 cores; no collectives.
"""

import math
import sys

import numpy as np

if "/opt/trn_rl_repo" not in sys.path:
    sys.path.insert(0, "/opt/trn_rl_repo")

B, H, T, N = 2, 16, 4096, 64
THETA = 2.0 ** 16
NCORES = 8
HPC = (B * H) // NCORES  # heads per core


def _host_tables(t_len):
    """Scaled, pair-deduplicated RoPE tables: C,S [t_len, 32] float32."""
    n = np.arange(N, dtype=np.float64)
    tq = np.floor(n / 2.0) * 2.0
    freqs = 1.0 / (THETA ** (tq / N)) / (2.0 * math.pi)  # [N]
    t = np.arange(t_len, dtype=np.float64)[:, None]
    ang = ((t * freqs[None, :]) % 1.0) * (2.0 * math.pi)  # [t_len, N]
    scale = float(N) ** -0.25
    cos = (np.cos(ang) * scale).astype(np.float32)
    sin = (np.sin(ang) * scale).astype(np.float32)
    # pairs share the table value (tq quantization): keep even columns
    return np.ascontiguousarray(cos[:, 0::2]), np.ascontiguousarray(sin[:, 0::2])


def build_program(t_len=T, hpc=HPC):
    import concourse.mybir as mybir
    import concourse.tile as tile
    from concourse import bacc

    f32 = mybir.dt.float32
    ch = t_len // 128  # number of 128-row chunks per head

    nc = bacc.Bacc(None, target_bir_lowering=False)
    q = nc.dram_tensor("q", [hpc, t_len, N], f32, kind="ExternalInput")
    v = nc.dram_tensor("v", [hpc, t_len, N], f32, kind="ExternalInput")
    ce = nc.dram_tensor("ce", [t_len, 32], f32, kind="ExternalInput")
    se = nc.dram_tensor("se", [t_len, 32], f32, kind="ExternalInput")
    mu = nc.dram_tensor("mu", [128, 128], f32, kind="ExternalInput")
    ident = nc.dram_tensor("ident", [128, 128], f32, kind="ExternalInput")
    o = nc.dram_tensor("o", [hpc, t_len, N], f32, kind="ExternalOutput")

    with tile.TileContext(nc) as tc:
        with (
            tc.tile_pool(name="const", bufs=1) as constp,
            tc.tile_pool(name="head", bufs=2) as headp,
            tc.tile_pool(name="work", bufs=3) as workp,
            tc.tile_pool(name="ps", bufs=2, space="PSUM") as psp,
        ):
            ce_sb = constp.tile([128, ch * 32], f32)
            se_sb = constp.tile([128, ch * 32], f32)
            mu_sb = constp.tile([128, 128], f32)
            id_sb = constp.tile([128, 128], f32)
            ce_v = ce_sb.rearrange("p (c m) -> p c m", c=ch)
            se_v = se_sb.rearrange("p (c m) -> p c m", c=ch)
            nc.sync.dma_start(ce_v, ce.rearrange("(c p) m -> p c m", p=128))
            nc.sync.dma_start(se_v, se.rearrange("(c p) m -> p c m", p=128))
            nc.sync.dma_start(mu_sb[:], mu[:])
            nc.sync.dma_start(id_sb[:], ident[:])

            for h in range(hpc):
                q_sb = headp.tile([128, ch * N], f32, tag="q")
                v_sb = headp.tile([128, ch * N], f32, tag="v")
                qrp = headp.tile([128, ch * N], f32, tag="qrp")
                qrpt = headp.tile([64, ch * 128], f32, tag="qrpt")
                sc1 = headp.tile([128, ch * 32], f32, tag="sc1")
                sc2 = headp.tile([128, ch * 32], f32, tag="sc2")

                nc.sync.dma_start(
                    q_sb.rearrange("p (c n) -> p c n", c=ch),
                    q[h].rearrange("(c p) n -> p c n", p=128),
                )
                nc.sync.dma_start(
                    v_sb.rearrange("p (c n) -> p c n", c=ch),
                    v[h].rearrange("(c p) n -> p c n", p=128),
                )

                # RoPE in permuted layout: qrp cols c*64+[0:32] = C*Qe - S*Qo,
                # cols c*64+[32:64] = S*Qe + C*Qo
                q4 = q_sb.rearrange("p (c m o) -> p c m o", c=ch, m=32, o=2)
                qe, qo = q4[:, :, :, 0], q4[:, :, :, 1]
                qr4 = qrp.rearrange("p (c g m) -> p c g m", c=ch, g=2, m=32)
                qre, qro = qr4[:, :, 0, :], qr4[:, :, 1, :]
                s1 = sc1.rearrange("p (c m) -> p c m", c=ch)
                s2 = sc2.rearrange("p (c m) -> p c m", c=ch)
                nc.vector.tensor_mul(s1, qe, ce_v)
                nc.vector.tensor_mul(s2, qo, se_v)
                nc.vector.tensor_sub(qre, s1, s2)
                nc.vector.tensor_mul(s1, qo, ce_v)
                nc.vector.tensor_mul(s2, qe, se_v)
                nc.vector.tensor_add(qro, s1, s2)

                m_prev = None
                for c in range(ch):
                    qrp_c = qrp[:, c * 64:(c + 1) * 64]
                    v_c = v_sb[:, c * 64:(c + 1) * 64]

                    # build QRP^T chunk via PE transpose
                    tr_ps = psp.tile([64, 128], f32, tag="tr")
                    nc.tensor.transpose(tr_ps[:], qrp_c, id_sb[:])
                    qrpt_c = qrpt[:, c * 128:(c + 1) * 128]
                    nc.scalar.copy(qrpt_c, tr_ps[:])

                    # intra: P = QRP_c @ QRP_c^T (symmetric diag block), then
                    # strict-upper mask (as lhsT for the second matmul)
                    p_ps = psp.tile([128, 128], f32, tag="p")
                    nc.tensor.matmul(p_ps[:], qrpt_c, qrpt_c, start=True, stop=True)
                    p_sb = workp.tile([128, 128], f32, tag="psb")
                    nc.vector.tensor_mul(p_sb[:], p_ps[:], mu_sb[:])

                    out_ps = psp.tile([128, 64], f32, tag="out")
                    if c == 0:
                        nc.tensor.matmul(out_ps[:], p_sb[:], v_c, start=True, stop=True)
                    else:
                        # inter: out += QRP_c @ M   (M = state after chunk c-1)
                        nc.tensor.matmul(
                            out_ps[:], qrpt_c, m_prev[:], start=True, stop=False
                        )
                        nc.tensor.matmul(
                            out_ps[:], p_sb[:], v_c, start=False, stop=True
                        )

                    # state: G = QRP_c^T @ V_c ; M += G (on DVE, keeps groups closed)
                    g_ps = psp.tile([64, 64], f32, tag="g")
                    nc.tensor.matmul(g_ps[:], qrp_c, v_c, start=True, stop=True)
                    m_sb = workp.tile([64, 64], f32, tag="msb")
                    if c == 0:
                        nc.vector.tensor_copy(m_sb[:], g_ps[:])
                    else:
                        nc.vector.tensor_add(m_sb[:], m_prev[:], g_ps[:])
                    m_prev = m_sb

                    out_sb = workp.tile([128, 64], f32, tag="osb")
                    nc.scalar.copy(out_sb[:], out_ps[:])
                    nc.sync.dma_start(o[h, c * 128:(c + 1) * 128, :], out_sb[:])

    nc.compile()
    return nc


_CACHE = {}


def _get_program():
    if "nc" not in _CACHE:
        _CACHE["nc"] = build_program()
    return _CACHE["nc"]


def _strict_upper_mask():
    # lhsT for the diag block: keep P[j, i] where j < i
    return np.triu(np.ones((128, 128), dtype=np.float32), k=1)


def kernel(Q, V):
    from concourse.bass_utils import run_bass_kernel_spmd

    Q = np.ascontiguousarray(np.asarray(Q), dtype=np.float32)
    V = np.ascontiguousarray(np.asarray(V), dtype=np.float32)
    qf = Q.reshape(NCORES, HPC, T, N)
    vf = V.reshape(NCORES, HPC, T, N)
    ce, se = _host_tables(T)
    mu = _strict_upper_mask()
    ident = np.eye(128, dtype=np.float32)

    nc = _get_program()
    in_maps = [
        {"q": qf[i], "v": vf[i], "ce": ce, "se": se, "mu": mu, "ident": ident}
        for i in range(NCORES)
    ]
    res = run_bass_kernel_spmd(nc, in_maps, core_ids=list(range(NCORES)))
    out = np.stack([r["o"] for r in res.results], axis=0)
    return out.reshape(B, H, T, N)


# revision 10
# speedup vs baseline: 1.9447x; 1.9447x over previous
"""Trainium2 Bass kernel for nn_Attention_23424751632639.

Computation (per (b,h)):  out = tril_strict(rope(Q) @ rope(Q).T / sqrt(N)) @ V
Reformulated as chunked linear attention (exact, just reordered sums):
  out_c = QRP_c @ M_c  +  strict_mask(QRP_c @ QRP_c^T) @ V_c
  M_{c+1} = M_c + QRP_c^T @ V_c            (M is the [64,64] running state)
with QRP = rope(Q) * N**-0.25 (scale folded into the cos/sin tables, so the
score scale N**-0.5 appears automatically in both the intra and inter terms).

Feature order inside the kernel is permuted to (all even feats, all odd feats)
so RoPE needs no cross-partition ops; the permutation is contraction-invariant
and never visible in the output.

Matmul operands are bf16 (PE runs 1 cyc/row vs 4 for fp32); all accumulation
(PSUM, the M state) stays fp32.  Set mm_dtype="f32" for the full-precision
variant.

Sharding: B*H = 32 (b,h) pairs -> 4 per core across 8 cores; no collectives.
"""

import math
import os
import sys

import numpy as np

if "/opt/trn_rl_repo" not in sys.path:
    sys.path.insert(0, "/opt/trn_rl_repo")

B, H, T, N = 2, 16, 4096, 64
THETA = 2.0 ** 16
NCORES = 8
HPC = (B * H) // NCORES  # heads per core

MM_DTYPE = os.environ.get("BASS_MM_DTYPE", "bf16")


def _host_tables(t_len):
    """Scaled, pair-deduplicated RoPE tables: C,S [t_len, 32] float32."""
    n = np.arange(N, dtype=np.float64)
    tq = np.floor(n / 2.0) * 2.0
    freqs = 1.0 / (THETA ** (tq / N)) / (2.0 * math.pi)  # [N]
    t = np.arange(t_len, dtype=np.float64)[:, None]
    ang = ((t * freqs[None, :]) % 1.0) * (2.0 * math.pi)  # [t_len, N]
    scale = float(N) ** -0.25
    cos = (np.cos(ang) * scale).astype(np.float32)
    sin = (np.sin(ang) * scale).astype(np.float32)
    # pairs share the table value (tq quantization): keep even columns
    return np.ascontiguousarray(cos[:, 0::2]), np.ascontiguousarray(sin[:, 0::2])


def build_program(t_len=T, hpc=HPC, mm_dtype=MM_DTYPE):
    import concourse.mybir as mybir
    import concourse.tile as tile
    from concourse import bacc

    f32 = mybir.dt.float32
    md = mybir.dt.bfloat16 if mm_dtype == "bf16" else f32
    ch = t_len // 128  # number of 128-row chunks per head

    nc = bacc.Bacc(None, target_bir_lowering=False)
    q = nc.dram_tensor("q", [hpc, t_len, N], f32, kind="ExternalInput")
    v = nc.dram_tensor("v", [hpc, t_len, N], f32, kind="ExternalInput")
    ce = nc.dram_tensor("ce", [t_len, 32], f32, kind="ExternalInput")
    se = nc.dram_tensor("se", [t_len, 32], f32, kind="ExternalInput")
    mu = nc.dram_tensor("mu", [128, 128], f32, kind="ExternalInput")
    ident = nc.dram_tensor("ident", [128, 128], md, kind="ExternalInput")
    o = nc.dram_tensor("o", [hpc, t_len, N], f32, kind="ExternalOutput")

    with tile.TileContext(nc) as tc:
        with (
            tc.tile_pool(name="const", bufs=1) as constp,
            tc.tile_pool(name="head", bufs=2) as headp,
            tc.tile_pool(name="work", bufs=3) as workp,
            tc.tile_pool(name="ps", bufs=2, space="PSUM") as psp,
        ):
            ce_sb = constp.tile([128, ch * 32], f32)
            se_sb = constp.tile([128, ch * 32], f32)
            mu_sb = constp.tile([128, 128], f32)
            id_sb = constp.tile([128, 128], md)
            ce_v = ce_sb.rearrange("p (c m) -> p c m", c=ch)
            se_v = se_sb.rearrange("p (c m) -> p c m", c=ch)
            nc.sync.dma_start(ce_v, ce.rearrange("(c p) m -> p c m", p=128))
            nc.sync.dma_start(se_v, se.rearrange("(c p) m -> p c m", p=128))
            nc.sync.dma_start(mu_sb[:], mu[:])
            nc.sync.dma_start(id_sb[:], ident[:])

            for h in range(hpc):
                q_sb = headp.tile([128, ch * N], f32, tag="q")
                v_sb = headp.tile([128, ch * N], md, tag="v")
                qrp = headp.tile([128, ch * N], md, tag="qrp")
                qrpt = headp.tile([64, ch * 128], md, tag="qrpt")
                sc1 = headp.tile([128, ch * 32], f32, tag="sc1")
                sc2 = headp.tile([128, ch * 32], f32, tag="sc2")

                nc.sync.dma_start(
                    q_sb.rearrange("p (c n) -> p c n", c=ch),
                    q[h].rearrange("(c p) n -> p c n", p=128),
                )
                # SWDGE casts f32 -> bf16 during the transfer
                veng = nc.gpsimd if mm_dtype == "bf16" else nc.sync
                veng.dma_start(
                    v_sb.rearrange("p (c n) -> p c n", c=ch),
                    v[h].rearrange("(c p) n -> p c n", p=128),
                )

                # RoPE in permuted layout: qrp cols c*64+[0:32] = C*Qe - S*Qo,
                # cols c*64+[32:64] = S*Qe + C*Qo
                q4 = q_sb.rearrange("p (c m o) -> p c m o", c=ch, m=32, o=2)
                qe, qo = q4[:, :, :, 0], q4[:, :, :, 1]
                qr4 = qrp.rearrange("p (c g m) -> p c g m", c=ch, g=2, m=32)
                qre, qro = qr4[:, :, 0, :], qr4[:, :, 1, :]
                s1 = sc1.rearrange("p (c m) -> p c m", c=ch)
                s2 = sc2.rearrange("p (c m) -> p c m", c=ch)
                nc.vector.tensor_mul(s1, qe, ce_v)
                nc.vector.tensor_mul(s2, qo, se_v)
                nc.vector.tensor_sub(qre, s1, s2)
                nc.vector.tensor_mul(s1, qo, ce_v)
                nc.vector.tensor_mul(s2, qe, se_v)
                nc.vector.tensor_add(qro, s1, s2)

                m_prev = None  # fp32 running state
                mb_prev = None  # bf16 rounded copy for the PE
                for c in range(ch):
                    qrp_c = qrp[:, c * 64:(c + 1) * 64]
                    v_c = v_sb[:, c * 64:(c + 1) * 64]

                    # build QRP^T chunk via PE transpose
                    tr_ps = psp.tile([64, 128], md, tag="tr")
                    nc.tensor.transpose(tr_ps[:], qrp_c, id_sb[:])
                    qrpt_c = qrpt[:, c * 128:(c + 1) * 128]
                    nc.scalar.copy(qrpt_c, tr_ps[:])

                    # intra: P = QRP_c @ QRP_c^T (symmetric diag block), then
                    # strict-upper mask (as lhsT for the second matmul)
                    p_ps = psp.tile([128, 128], f32, tag="p")
                    nc.tensor.matmul(p_ps[:], qrpt_c, qrpt_c, start=True, stop=True)
                    p_sb = workp.tile([128, 128], md, tag="psb")
                    nc.vector.tensor_mul(p_sb[:], p_ps[:], mu_sb[:])

                    out_ps = psp.tile([128, 64], f32, tag="out")
                    if c == 0:
                        nc.tensor.matmul(out_ps[:], p_sb[:], v_c, start=True, stop=True)
                    else:
                        # inter: out += QRP_c @ M   (M = state after chunk c-1)
                        nc.tensor.matmul(
                            out_ps[:], qrpt_c, mb_prev[:], start=True, stop=False
                        )
                        nc.tensor.matmul(
                            out_ps[:], p_sb[:], v_c, start=False, stop=True
                        )

                    # state: G = QRP_c^T @ V_c ; M += G (fp32 on DVE)
                    g_ps = psp.tile([64, 64], f32, tag="g")
                    nc.tensor.matmul(g_ps[:], qrp_c, v_c, start=True, stop=True)
                    m_sb = workp.tile([64, 64], f32, tag="msb")
                    if c == 0:
                        nc.vector.tensor_copy(m_sb[:], g_ps[:])
                    else:
                        nc.vector.tensor_add(m_sb[:], m_prev[:], g_ps[:])
                    m_prev = m_sb
                    if md is not f32:
                        m_bf = workp.tile([64, 64], md, tag="mbf")
                        nc.vector.tensor_copy(m_bf[:], m_sb[:])
                        mb_prev = m_bf
                    else:
                        mb_prev = m_sb

                    out_sb = workp.tile([128, 64], f32, tag="osb")
                    nc.scalar.copy(out_sb[:], out_ps[:])
                    nc.sync.dma_start(o[h, c * 128:(c + 1) * 128, :], out_sb[:])

    nc.compile()
    return nc


_CACHE = {}


def _get_program():
    if "nc" not in _CACHE:
        _CACHE["nc"] = build_program()
    return _CACHE["nc"]


def _strict_upper_mask():
    # lhsT for the diag block: keep P[j, i] where j < i
    return np.triu(np.ones((128, 128), dtype=np.float32), k=1)


def _identity(mm_dtype=MM_DTYPE):
    if mm_dtype == "bf16":
        import ml_dtypes

        return np.eye(128, dtype=ml_dtypes.bfloat16)
    return np.eye(128, dtype=np.float32)


def kernel(Q, V):
    from concourse.bass_utils import run_bass_kernel_spmd

    Q = np.ascontiguousarray(np.asarray(Q), dtype=np.float32)
    V = np.ascontiguousarray(np.asarray(V), dtype=np.float32)
    qf = Q.reshape(NCORES, HPC, T, N)
    vf = V.reshape(NCORES, HPC, T, N)
    ce, se = _host_tables(T)
    mu = _strict_upper_mask()
    ident = _identity()

    nc = _get_program()
    in_maps = [
        {"q": qf[i], "v": vf[i], "ce": ce, "se": se, "mu": mu, "ident": ident}
        for i in range(NCORES)
    ]
    res = run_bass_kernel_spmd(nc, in_maps, core_ids=list(range(NCORES)))
    out = np.stack([r["o"] for r in res.results], axis=0)
    return out.reshape(B, H, T, N)
